# revision 12
# baseline (speedup 1.0000x reference)
"""Two-layer DenseGAT (H=1) on 8 Trainium2 NeuronCores.

Strategy (per core, rows sharded 8-way, everything in "transposed"
[j-partition, i-free] orientation so the attention matmul contracts on
partitions):

  - h = x @ W on PE in f32 (lhsT = xT blocks), stored fp16 for the PV matmul.
  - a_src (all nodes) comes for free from the h PSUM tiles via a DVE
    tensor_tensor_reduce against a replicated att_src; a_dst (own rows) from a
    tiny PE matmul; split hi/lo into fp16 so a K=3 fp16 matmul rebuilds it
    exactly in f32 PSUM.
  - mask folded into PSUM accumulation: z'' = (d_hi + d_lo - BIG) + BIG*adjT,
    where BIG*adjT comes from a PE "transpose matmul" (lhsT = adj block,
    rhs = BIG*I).  adj is cast f32->fp16 during the SWDGE DMA load.
  - ScalarE: t = Prelu(z'' + a_src[j], alpha=0.2); e = exp(t - 8) in fp16.
    Masked entries have t ~ -100 so exp underflows to exactly 0; the -8 shift
    keeps e in fp16 range and cancels in the softmax ratio.
  - PV: out^T += h_tile^T @ e on PE (fp16 operands, f32 PSUM); denominator via
    DVE running accumulation + one GPSIMD cross-partition reduce.
  - Layer 1 output is relu'd, AllGathered (DRAM collective) to form layer 2's
    x^T; outputs returned transposed, host transposes back.
"""

import sys

import numpy as np


def _ensure_path():
    try:
        import concourse.bass  # noqa: F401
    except ImportError:
        for p in ("/opt/trn_rl_repo", "/root/.axon_site/_ro/trn_rl_repo"):
            if p not in sys.path:
                sys.path.insert(0, p)


N_CORES = 8
BIG = 512.0
EXP_SHIFT = -8.0
ALPHA = 0.2  # DenseGATConv negative_slope

_prog_cache = {}


def build_program(N=8192, D=128, jchunk=16, use_collective=True):
    """Build (and cache) the SPMD Bass program for all 8 cores."""
    key = (N, D, jchunk, use_collective)
    if key in _prog_cache:
        return _prog_cache[key]
    _ensure_path()
    from contextlib import ExitStack

    import concourse.tile as tile
    from concourse import bacc, mybir

    dt = mybir.dt
    f32, f16 = dt.float32, dt.float16
    AF = mybir.ActivationFunctionType
    ALU = mybir.AluOpType
    AX = mybir.AxisListType

    R = N // N_CORES          # rows per core
    NJT = N // 128            # j tiles
    NB = R // 128             # 128-row blocks per core
    nchunks = NJT // jchunk   # adj DMA chunks per layer
    # PV / z free-dim split into <=512 pieces (f32 PSUM bank limit)
    nsplit = max(1, R // 512)
    seg = R // nsplit

    nc = bacc.Bacc("TRN2", target_bir_lowering=False, debug=False,
                   num_devices=N_CORES)

    adjshard = nc.dram_tensor("adjshard", [R, N], f32, kind="ExternalInput")
    xT_d = nc.dram_tensor("xT", [D, N], f32, kind="ExternalInput")
    xTown_d = nc.dram_tensor("xTown", [D, R], f32, kind="ExternalInput")
    W_d = [nc.dram_tensor(f"W{L}", [D, D], f32, kind="ExternalInput")
           for L in range(2)]
    attw_d = [nc.dram_tensor(f"attw{L}", [D, 1], f32, kind="ExternalInput")
              for L in range(2)]
    attrep_d = [nc.dram_tensor(f"attrep{L}", [128, D], f32, kind="ExternalInput")
                for L in range(2)]
    bcol_d = [nc.dram_tensor(f"bcol{L}", [D, 1], f32, kind="ExternalInput")
              for L in range(2)]
    zconst_d = nc.dram_tensor("zconst", [3, 128], f16, kind="ExternalInput")
    diag_d = nc.dram_tensor("diagBIG", [128, 128], f16, kind="ExternalInput")
    ones_d = nc.dram_tensor("onescol", [1, 128], f32, kind="ExternalInput")
    outT_d = [nc.dram_tensor(f"out{L}T", [D, R], f32, kind="ExternalOutput")
              for L in range(2)]

    with tile.TileContext(nc) as tc, ExitStack() as ctx:
        const = ctx.enter_context(tc.tile_pool(name="const", bufs=1))
        xpool = ctx.enter_context(tc.tile_pool(name="xpool", bufs=1))
        adjp = ctx.enter_context(tc.tile_pool(name="adjp", bufs=2))
        hpool = ctx.enter_context(tc.tile_pool(name="hpool", bufs=1))
        tpool = ctx.enter_context(tc.tile_pool(name="tpool", bufs=3))
        epool = ctx.enter_context(tc.tile_pool(name="epool", bufs=3))
        spool = ctx.enter_context(tc.tile_pool(name="spool", bufs=2))
        dpool = ctx.enter_context(tc.tile_pool(name="dpool", bufs=1))
        drmp = ctx.enter_context(tc.tile_pool(name="drmp", bufs=1, space="DRAM"))
        zps = ctx.enter_context(tc.tile_pool(name="zps", bufs=2, space="PSUM"))
        gps = ctx.enter_context(tc.tile_pool(name="gps", bufs=1, space="PSUM"))
        sps = ctx.enter_context(tc.tile_pool(name="sps", bufs=2, space="PSUM"))

        # ---- constants into SBUF
        W_sb, attw_sb, attrep_sb, bcol_sb = [], [], [], []
        for L in range(2):
            w = const.tile([D, D], f32, name=f"w{L}_sb")
            nc.sync.dma_start(w[:], W_d[L][:])
            W_sb.append(w)
            aw = const.tile([D, 1], f32, name=f"attw{L}_sb")
            nc.sync.dma_start(aw[:], attw_d[L][:])
            attw_sb.append(aw)
            ar = const.tile([128, D], f32, name=f"attrep{L}_sb")
            nc.sync.dma_start(ar[:], attrep_d[L][:])
            attrep_sb.append(ar)
            bc = const.tile([D, 1], f32, name=f"bcol{L}_sb")
            nc.sync.dma_start(bc[:], bcol_d[L][:])
            bcol_sb.append(bc)
        zconst_sb = const.tile([3, 128], f16)
        nc.sync.dma_start(zconst_sb[:], zconst_d[:])
        diag_sb = const.tile([128, 128], f16)
        nc.sync.dma_start(diag_sb[:], diag_d[:])
        ones_sb = const.tile([1, 128], f32)
        nc.sync.dma_start(ones_sb[:], ones_d[:])
        expb_sb = const.tile([128, 1], f32)
        nc.vector.memset(expb_sb[:], EXP_SHIFT)
        xTown_sb = const.tile([D, R], f32)
        nc.sync.dma_start(xTown_sb[:], xTown_d[:])

        ag_in = drmp.tile([D, R], f32, name="ag_in")
        ag_out = drmp.tile([N_CORES, D, R], f32, name="ag_out")

        own_prev = None
        for L in range(2):
            Wl, attwl, attrepl = W_sb[L], attw_sb[L], attrep_sb[L]
            if L == 0:
                xTfull = xpool.tile([128, N], f32, tag="xfull", name="xT0_sb")
                nc.sync.dma_start(xTfull[:], xT_d[:])
                xTown_cur = xTown_sb
            else:
                xTfull = xpool.tile([128, N], f32, tag="xfull", name="xT1_sb")
                nc.sync.dma_start(
                    xTfull[:].rearrange("p (g i) -> p g i", g=N_CORES),
                    ag_out.rearrange("g p i -> p g i"))
                xTown_cur = own_prev

            # ---- prologue: h (fp16) + a_src column layout (f32, exact)
            h_sb = hpool.tile([128, NJT * 128], f16, tag="h", name=f"h{L}_sb")
            ascol = dpool.tile([128, NJT], f32, tag="ascol", name=f"ascol{L}")
            for nb in range(NJT):
                hp = sps.tile([128, 128], f32, tag="sp", name=f"hp{L}_{nb}")
                nc.tensor.matmul(hp[:], xTfull[:, nb * 128:(nb + 1) * 128],
                                 Wl[:], start=True, stop=True)
                nc.vector.tensor_copy(h_sb[:, nb * 128:(nb + 1) * 128], hp[:])
                junk = spool.tile([128, 128], f32, tag="junk",
                                  name=f"junk{L}_{nb}")
                nc.vector.tensor_mul(junk[:], hp[:], attrepl[:])
                nc.vector.tensor_reduce(ascol[:, nb:nb + 1], junk[:],
                                        axis=AX.X, op=ALU.add)

            # ---- a_dst for own rows -> dstones rows (hi/lo fp16 split).
            # DVE may only address partition bases 0/32/64/96, so each row is
            # computed in its own [1, R] tile and placed into the [3, R]
            # matmul operand with SBUF->SBUF DMAs.
            adst = dpool.tile([1, R], f32, tag="adst", name=f"adst{L}")
            for s in range(nsplit):
                adp = sps.tile([1, seg], f32, tag="sp", name=f"adp{L}_{s}")
                nc.tensor.matmul(adp[:], attwl[:],
                                 xTown_cur[:, s * seg:(s + 1) * seg],
                                 start=True, stop=True)
                nc.vector.tensor_copy(adst[:, s * seg:(s + 1) * seg], adp[:])
            dstones = dpool.tile([3, R], f16, tag="dstones", name=f"dst{L}")
            dhi = dpool.tile([1, R], f16, tag="dhi", name=f"dhi{L}")
            dlo = dpool.tile([1, R], f16, tag="dlo", name=f"dlo{L}")
            onesr = dpool.tile([1, R], f16, tag="onesr", name=f"onesr{L}")
            tmpa = dpool.tile([1, R], f32, tag="tmpa", name=f"tmpa{L}")
            tmpb = dpool.tile([1, R], f32, tag="tmpb", name=f"tmpb{L}")
            nc.vector.tensor_copy(dhi[:], adst[:])               # d_hi (f16)
            nc.vector.tensor_copy(tmpa[:], dhi[:])               # back to f32
            nc.vector.tensor_sub(tmpb[:], adst[:], tmpa[:])
            nc.vector.tensor_copy(dlo[:], tmpb[:])               # d_lo (f16)
            nc.vector.memset(onesr[:], 1.0)
            nc.sync.dma_start(dstones[0:1, :], dhi[:])
            nc.sync.dma_start(dstones[1:2, :], dlo[:])
            nc.sync.dma_start(dstones[2:3, :], onesr[:])

            pacc = dpool.tile([128, R], f32, tag="pacc", name=f"pacc{L}")
            nc.vector.memset(pacc[:], 0.0)
            g_ps = gps.tile([128, R], f32, tag="g", name=f"g{L}_ps")

            # ---- main attention loop over j tiles
            for jc in range(nchunks):
                adjt = adjp.tile([128, NB, jchunk * 128], f16, tag="adj",
                                 name=f"adj{L}_{jc}")
                src = adjshard[:].rearrange("(b p) j -> p b j", p=128)[
                    :, :, jc * jchunk * 128:(jc + 1) * jchunk * 128]
                nc.gpsimd.dma_start(adjt[:], src)  # f32 -> fp16 cast in DMA
                for jl in range(jchunk):
                    j = jc * jchunk + jl
                    z_ps = zps.tile([128, R], f32, tag="z", name=f"z{L}_{j}")
                    for s in range(nsplit):
                        nc.tensor.matmul(
                            z_ps[:, s * seg:(s + 1) * seg], zconst_sb[:],
                            dstones[:, s * seg:(s + 1) * seg],
                            start=True, stop=False)
                    for b in range(NB):
                        nc.tensor.matmul(
                            z_ps[:, b * 128:(b + 1) * 128],
                            adjt[:, b, jl * 128:(jl + 1) * 128],
                            diag_sb[:], start=False, stop=True)
                    t_sb = tpool.tile([128, R], f32, tag="t", name=f"t{L}_{j}")
                    nc.scalar.activation(t_sb[:], z_ps[:], AF.Prelu,
                                         bias=ascol[:, j:j + 1], scale=1.0,
                                         alpha=ALPHA)
                    e_sb = epool.tile([128, R], f16, tag="e", name=f"e{L}_{j}")
                    nc.scalar.activation(e_sb[:], t_sb[:], AF.Exp,
                                         bias=expb_sb[:], scale=1.0)
                    nc.vector.tensor_tensor(pacc[:], pacc[:], e_sb[:], ALU.add)
                    for s in range(nsplit):
                        nc.tensor.matmul(
                            g_ps[:, s * seg:(s + 1) * seg],
                            h_sb[:, j * 128:(j + 1) * 128],
                            e_sb[:, s * seg:(s + 1) * seg],
                            start=(j == 0), stop=(j == NJT - 1))

            # ---- normalize + bias (+relu on layer 0)
            drow = dpool.tile([1, R], f32, tag="drow", name=f"drow{L}")
            nc.gpsimd.tensor_reduce(drow[:], pacc[:], axis=AX.C, op=ALU.add)
            rrow = dpool.tile([1, R], f32, tag="rrow", name=f"rrow{L}")
            nc.vector.reciprocal(rrow[:], drow[:])
            rdb = dpool.tile([128, R], f32, tag="rdb", name=f"rdb{L}")
            for s in range(nsplit):
                bp = sps.tile([128, seg], f32, tag="sp", name=f"bp{L}_{s}")
                nc.tensor.matmul(bp[:], ones_sb[:],
                                 rrow[:, s * seg:(s + 1) * seg],
                                 start=True, stop=True)
                nc.vector.tensor_copy(rdb[:, s * seg:(s + 1) * seg], bp[:])
            od = dpool.tile([128, R], f32, tag="od", name=f"od{L}")
            nc.vector.tensor_mul(od[:], g_ps[:], rdb[:])
            outT_sb = dpool.tile([128, R], f32, tag=f"outT{L}",
                                 name=f"outT{L}_sb")
            if L == 0:
                nc.scalar.activation(outT_sb[:], od[:], AF.Relu,
                                     bias=bcol_sb[0][:], scale=1.0)
                nc.sync.dma_start(outT_d[0][:], outT_sb[:])
                nc.sync.dma_start(ag_in[:], outT_sb[:])
                if use_collective:
                    nc.gpsimd.collective_compute(
                        "AllGather", ALU.bypass,
                        replica_groups=[list(range(N_CORES))],
                        ins=[ag_in.opt()], outs=[ag_out.opt()])
                else:
                    for g in range(N_CORES):
                        nc.sync.dma_start(ag_out[g], ag_in[:])
                own_prev = outT_sb
            else:
                nc.vector.tensor_scalar_add(outT_sb[:], od[:], bcol_sb[1][:])
                nc.sync.dma_start(outT_d[1][:], outT_sb[:])

    nc.compile()
    _prog_cache[key] = nc
    return nc


def make_in_maps(x, adj, W0, att_src0, att_dst0, b0, W1, att_src1, att_dst1,
                 b1, N=8192, D=128):
    R = N // N_CORES
    f32 = np.float32
    x0 = np.asarray(x, f32).reshape(N, D)
    adj0 = np.asarray(adj, f32).reshape(N, N)
    xT = np.ascontiguousarray(x0.T)
    base = {
        "xT": xT,
        "W0": np.asarray(W0, f32),
        "W1": np.asarray(W1, f32),
        "attw0": np.ascontiguousarray(
            (np.asarray(W0, f32) @ np.asarray(att_dst0, f32)[0])[:, None]),
        "attw1": np.ascontiguousarray(
            (np.asarray(W1, f32) @ np.asarray(att_dst1, f32)[0])[:, None]),
        "attrep0": np.ascontiguousarray(
            np.repeat(np.asarray(att_src0, f32), 128, axis=0)),
        "attrep1": np.ascontiguousarray(
            np.repeat(np.asarray(att_src1, f32), 128, axis=0)),
        "bcol0": np.asarray(b0, f32).reshape(D, 1),
        "bcol1": np.asarray(b1, f32).reshape(D, 1),
        "zconst": np.ascontiguousarray(np.stack(
            [np.ones(128), np.ones(128), -BIG * np.ones(128)]
        ).astype(np.float16)),
        "diagBIG": np.ascontiguousarray(
            (BIG * np.eye(128)).astype(np.float16)),
        "onescol": np.ones((1, 128), f32),
    }
    in_maps = []
    for k in range(N_CORES):
        m = dict(base)
        m["adjshard"] = np.ascontiguousarray(adj0[k * R:(k + 1) * R])
        m["xTown"] = np.ascontiguousarray(xT[:, k * R:(k + 1) * R])
        in_maps.append(m)
    return in_maps


def _install_ntff_shim():
    """Register the missing antenv.axon_hooks module so trace=True works."""
    import types
    try:
        from antenv import axon_hooks  # noqa: F401
        return
    except ImportError:
        pass
    try:
        import antenv
        from trn_agent_boot.trn_boot import _ntff_profile_via_ctypes
        hook = _ntff_profile_via_ctypes("/opt/axon/libaxon_pjrt.so")
        mod = types.ModuleType("antenv.axon_hooks")
        mod.get_axon_ntff_profile_hook = lambda: hook
        sys.modules["antenv.axon_hooks"] = mod
        antenv.axon_hooks = mod
    except Exception:
        pass


def run(inputs, trace=False, **trace_kwargs):
    _ensure_path()
    if trace:
        _install_ntff_shim()
    from concourse.bass_utils import run_bass_kernel_spmd

    x = inputs["x"]
    N, D = int(np.asarray(x).shape[1]), int(np.asarray(x).shape[2])
    nc = build_program(N=N, D=D, jchunk=min(16, N // 128))
    in_maps = make_in_maps(
        x, inputs["adj"], inputs["W0"], inputs["att_src0"],
        inputs["att_dst0"], inputs["b0"], inputs["W1"], inputs["att_src1"],
        inputs["att_dst1"], inputs["b1"], N=N, D=D)
    res = run_bass_kernel_spmd(nc, in_maps, list(range(N_CORES)),
                               trace=trace, **trace_kwargs)
    R = N // N_CORES
    out0 = np.concatenate(
        [np.asarray(res.results[k]["out0T"]).T for k in range(N_CORES)], 0)
    out1 = np.concatenate(
        [np.asarray(res.results[k]["out1T"]).T for k in range(N_CORES)], 0)
    return (out0, out1), res


def kernel(x, adj, W0, att_src0, att_dst0, b0, W1, att_src1, att_dst1, b1):
    outs, _ = run(dict(x=x, adj=adj, W0=W0, att_src0=att_src0,
                       att_dst0=att_dst0, b0=b0, W1=W1, att_src1=att_src1,
                       att_dst1=att_dst1, b1=b1))
    return outs
